# revision 1
# baseline (speedup 1.0000x reference)
"""Trainium2 SPMD kernel for nn_CombinedGeneModel.

Math (per batch b, tech t, gene g; R = T*G independent tiny MLPs):
    h   = relu(x * w1[r,e] + b1[r,e])          e = 0..3
    s   = relu(sum_e h*w2[r,e] + b2[r])
    out = relu(sum_t s[b,t,g]*wg[g,t] + bg[g])

With b1 == 0 (guaranteed by setup_inputs) the E=4 hinge sum folds exactly:
    sum_e w2_e*relu(w1_e*x) = c*relu(x) + d*x
      c = sum_e w2_e*|w1_e|,  d = sum_e w2_e*min(w1_e, 0)
so per row:  s = relu(c*relu(x) + d*x + b2).

Layout: genes on SBUF partitions, batch on the free axis; genes sharded
across the 8 NeuronCores; host pre-transposes x to [G, T, B] fp16 so all
DMA is contiguous.

Engine split (per 128-gene tile, free dim = 1024 batch):
  GpSimd : p = relu(x)  (both techs, one [128,2048] op)
  TensorE: u_t = diag(c_t) @ p_t + diag(d_t) @ x_t  -> PSUM (fp32)
           (idle otherwise; diagonal stationaries built on host)
  ScalarE: s_t = relu(u_t + b2_t)  PSUM -> SBUF fp16
  VectorE: out = relu(wg0*s0 + wg1*s1 + bg) -> output staging buffer
  GpSimd : 2 chunked SWDGE stores

The walrus build here accepts at most ONE sync wait per instruction, so the
kernel is arranged to keep every instruction at <=1 wait: no-reuse buffers
for all DMA targets, tiny same-engine "touch"/dummy-ldweights ops to absorb
extra semaphore waits, and a post-pass that splits the epilogue Drain.
"""

import os
import numpy as np

N_GENES = 20000
N_TECH = 2
BATCH = 1024
N_CORES = 8
P = 128
G_PAD = 20480            # next multiple of 8*128 above 20000
GS = G_PAD // N_CORES    # 2560 genes per core
NTILES = GS // P         # 20 tiles of 128 genes
FD = BATCH               # free dim per (tile, tech)
HF = FD // 2             # matmul moving-free-dim limit is 512
NCOL = 5                 # per-gene scalars: wg0, wg1, bg, b20, b21
STORE_CHUNK = 10         # 2 output stores -> 2 virgin SWDGE lanes

LAST_EXEC_NS = None
LAST_RESULTS = None

_nc_cache = {}


def _build_nc(has_b2: bool):
    import concourse.bass as bass
    import concourse.mybir as mybir
    from concourse.tile import TileContext

    Op = mybir.AluOpType
    Act = mybir.ActivationFunctionType
    f16 = mybir.dt.float16
    f32 = mybir.dt.float32

    nc = bass.Bass()
    x_d = nc.declare_dram_parameter("x", [NTILES, P, 2 * FD], f16, isOutput=False)
    w_d = nc.declare_dram_parameter("w", [P, NTILES * NCOL], f32, isOutput=False)
    g_d = nc.declare_dram_parameter("dg", [P, NTILES * 4 * P], f16, isOutput=False)
    o_d = nc.declare_dram_parameter("out", [NTILES, P, FD], f16, isOutput=True)

    with TileContext(nc) as tc:
        with (
            tc.tile_pool(name="wp", bufs=1) as wpool,
            tc.tile_pool(name="xp", bufs=NTILES) as xpool,
            tc.tile_pool(name="op", bufs=1) as opool,
            tc.tile_pool(name="pp", bufs=4) as ppool,
            tc.tile_pool(name="sp", bufs=4) as spool,
            tc.tile_pool(name="tp", bufs=3) as tpool,
            tc.tile_pool(name="sc", bufs=NTILES) as scpool,
            tc.tile_pool(name="ps", bufs=2, space="PSUM") as pspool,
        ):
            obuf = opool.tile([P, NTILES * FD], f16)
            w = wpool.tile([P, NTILES * NCOL], f32)
            nc.sync.dma_start(w[:], w_d[:])
            # diag load split: a small head chunk lands before the x loads
            # so the first tiles' matmuls are not gated on the full 2.6MB
            dg = wpool.tile([P, NTILES * 4 * P], f16)
            DG_HEAD = 3 * 4 * P
            nc.sync.dma_start(dg[:, 0:DG_HEAD], g_d[:, 0:DG_HEAD])
            nc.sync.dma_start(dg[:, DG_HEAD:], g_d[:, DG_HEAD:])

            # absorb the w/dg DMA waits once per consuming engine
            wt_v = wpool.tile([P, 1], f32)
            nc.vector.tensor_copy(wt_v[:], w[:, 0:1])
            wt_a = wpool.tile([P, 1], f32)
            nc.scalar.copy(wt_a[:], w[:, 0:1])
            nc.tensor.ldweights(dg[:, 0:P])       # PE touch: head diag chunk
            nc.tensor.ldweights(dg[:, DG_HEAD : DG_HEAD + P])  # PE touch: tail chunk

            xts = []
            sbs = []
            for j in range(NTILES):
                col = j * NCOL
                wg0 = w[:, col + 0 : col + 1]
                wg1 = w[:, col + 1 : col + 2]
                bg = w[:, col + 2 : col + 3]
                b20 = w[:, col + 3 : col + 4]
                b21 = w[:, col + 4 : col + 5]
                dgc = j * 4 * P
                dg_c0 = dg[:, dgc + 0 * P : dgc + 1 * P]
                dg_d0 = dg[:, dgc + 1 * P : dgc + 2 * P]
                dg_c1 = dg[:, dgc + 2 * P : dgc + 3 * P]
                dg_d1 = dg[:, dgc + 3 * P : dgc + 4 * P]

                xt = xpool.tile([P, 2 * FD], f16, tag="x")
                nc.sync.dma_start(xt[:], x_d[j])
                xts.append(xt)

                # DVE: absorb this load's DMA-lane wait, then p = relu(x)
                scx = scpool.tile([P, 1], f16, tag="scx")
                nc.vector.tensor_copy(scx[:], xt[:, 0:1])
                pf = ppool.tile([P, 2 * FD], f16, tag="p")
                nc.vector.tensor_scalar(pf[:], xt[:], 0.0, None, Op.max)

                # TensorE: u_t = diag(c_t)@p_t + diag(d_t)@x_t in PSUM.
                # dummy load_weights absorbs the GpSimd(p-ready) wait.
                nc.tensor.ldweights(pf[:, 0:P])
                # one PSUM tile per batch-half; layout [tech, HF] so ACT can
                # start the relu of a half after 4 matmuls instead of 8
                uhs = []
                for h in range(2):
                    uh = pspool.tile([P, 2 * HF], f32, tag=f"u{h}")
                    s0 = slice(h * HF, (h + 1) * HF)
                    s1 = slice(FD + h * HF, FD + (h + 1) * HF)
                    nc.tensor.matmul(uh[:, 0:HF], dg_c0, pf[:, s0],
                                     start=True, stop=False)
                    nc.tensor.matmul(uh[:, 0:HF], dg_d0, xt[:, s0],
                                     start=False, stop=True)
                    nc.tensor.matmul(uh[:, HF : 2 * HF], dg_c1, pf[:, s1],
                                     start=True, stop=False)
                    nc.tensor.matmul(uh[:, HF : 2 * HF], dg_d1, xt[:, s1],
                                     start=False, stop=True)
                    uhs.append(uh)

                # ScalarE: s_t = relu(u_t + b2_t), PSUM -> SBUF fp16.
                # scribble on the outgoing s-tile first: the tiny write
                # carries the WAR wait (DVE readers of sb[j-3]) and its WAW
                # ordering pins it before the relu that reuses the slot.
                if j >= 4:
                    nc.scalar.copy(sbs[j - 4][:, 0:1], wt_a[:])
                sb = spool.tile([P, 2 * FD], f16, tag="s")
                sbs.append(sb)
                for h in range(2):
                    # dest: (s0 half-h, s1 half-h) strided into sb
                    dst = sb[:].rearrange("p (t b) -> p t b", b=FD)[
                        :, :, h * HF : (h + 1) * HF
                    ]
                    if has_b2:
                        nc.scalar.activation(dst[:, 0:1, :], uhs[h][:, 0:HF],
                                             Act.Relu, bias=b20)
                        nc.scalar.activation(dst[:, 1:2, :], uhs[h][:, HF : 2 * HF],
                                             Act.Relu, bias=b21)
                    else:
                        nc.scalar.activation(dst, uhs[h][:], Act.Relu)

                # VectorE: out = relu(wg0*s0 + wg1*s1 + bg)
                t1 = tpool.tile([P, FD], f16, tag="t1")
                nc.vector.tensor_scalar(t1[:], sb[:, FD : 2 * FD], wg1, bg,
                                        Op.mult, Op.add)
                t0 = tpool.tile([P, FD], f16, tag="t0")
                nc.vector.tensor_scalar(t0[:], sb[:, 0:FD], wg0, None, Op.mult)
                ov = tpool.tile([P, FD], f16, tag="ov")
                nc.vector.tensor_tensor(ov[:], t0[:], t1[:], Op.add)
                ot = obuf[:, j * FD : (j + 1) * FD]
                nc.vector.tensor_scalar(ot, ov[:], 0.0, None, Op.max)

                if (j + 1) % STORE_CHUNK == 0:
                    k0 = j + 1 - STORE_CHUNK
                    src = obuf[:, k0 * FD : (j + 1) * FD].rearrange(
                        "p (t b) -> p t b", t=STORE_CHUNK
                    )
                    dst = o_d[k0 : j + 1].rearrange("t p b -> p t b")
                    nc.gpsimd.dma_start(dst, src)

    _split_multi_waits(nc, mybir)
    return nc


def _split_multi_waits(nc, mybir):
    """walrus (gen3 codegen here) accepts at most one sync wait per
    instruction.  Two rewrites keep every instruction at <=1 wait:

    1. Drop self-engine waits that are provably satisfied: engines run
       their stream in order and bump their own semaphore once per
       retired instruction, so a wait on the engine's own semaphore for
       a value already reached earlier in its own stream is a no-op
       (Tile emits these because its clock tracking is not transitive).
    2. For the remaining multi-wait instructions (the epilogue Drain,
       which is block-initial), hoist all but one wait onto same-engine
       NoOps appended to the preceding basic block."""
    blocks = list(nc.main_func.blocks)

    # sem id -> set of engines that increment it
    updaters = {}
    for bb in blocks:
        for ins in bb.instructions:
            si = getattr(ins, "sync_info", None)
            if si is None:
                continue
            for up in si.on_update or []:
                updaters.setdefault(up.id, set()).add(ins.engine)

    # pass 1: strip satisfied self-waits, walking in block order while
    # accumulating each semaphore's increments
    cum = {}
    for bb in blocks:
        for ins in bb.instructions:
            si = getattr(ins, "sync_info", None)
            if si is None:
                continue
            waits = list(si.on_wait or [])
            if len(waits) > 1:
                kept = []
                for wv in waits:
                    if (
                        wv.sync_type == "semaphore"
                        and wv.wait_mode == "sem-ge-imm"
                        and updaters.get(wv.id) == {ins.engine}
                        # engine sems increment at in-order instruction
                        # retirement, so earlier-stream increments prove the
                        # wait satisfied; DMA lane sems (DMAHW*/DMASW*)
                        # increment at async DMA *completion* — never strip
                        and "DMA" not in (wv.ant_name or "")
                        and cum.get(wv.id, 0) >= wv.wait_value
                    ):
                        continue  # provably satisfied self-wait
                    kept.append(wv)
                if len(kept) != len(waits):
                    ins.sync_info = mybir.SyncInfo(
                        on_wait=kept, on_update=list(si.on_update or [])
                    )
            si = ins.sync_info
            for up in si.on_update or []:
                if up.update_mode == "sem-inc":
                    cum[up.id] = cum.get(up.id, 0) + up.update_value

    # pass 2: NoOp-split anything still multi-wait (the Drain)
    nop_idx = 0
    for bi, bb in enumerate(blocks):
        for ins in bb.instructions:
            si = getattr(ins, "sync_info", None)
            if si is None:
                continue
            waits = list(si.on_wait or [])
            if len(waits) <= 1:
                continue
            assert bi > 0, f"multi-wait instruction in first block: {ins.name}"
            for other in bb.instructions:
                if other.name == ins.name:
                    break
                assert other.engine != ins.engine, (
                    f"cannot NoOp-split mid-block instruction {ins.name}"
                )
            prev_bb = blocks[bi - 1]
            for wv in waits[:-1]:
                nop = mybir.InstNoOp(name=f"ant-waitsplit-{nop_idx}")
                nop_idx += 1
                nop.engine = ins.engine
                nop.sync_info = mybir.SyncInfo(on_wait=[wv], on_update=[])
                prev_bb.add_instruction(nop)
            ins.sync_info = mybir.SyncInfo(
                on_wait=[waits[-1]], on_update=list(si.on_update or [])
            )


def _numpy_fallback(x, w1, b1, w2, b2, wg, bgv):
    B = x.shape[0]
    R = N_GENES * N_TECH
    xr = x.reshape(B, R).T.astype(np.float32)
    h = np.maximum(xr[:, :, None] * w1[:, None, :] + b1[:, None, :], 0.0)
    s = np.maximum(np.einsum("rbe,re->rb", h, w2) + b2[:, None], 0.0)
    s = s.T.reshape(B, N_TECH, N_GENES)
    out = np.maximum(np.einsum("btg,gt->bg", s, wg) + bgv, 0.0)
    return out.astype(np.float32)


def kernel(x, weights1, bias1, weights2, bias2, weights_g, bias_g):
    global LAST_EXEC_NS, LAST_RESULTS
    x = np.asarray(x, dtype=np.float32)
    w1 = np.asarray(weights1, dtype=np.float32)
    b1 = np.asarray(bias1, dtype=np.float32)
    w2 = np.asarray(weights2, dtype=np.float32)
    b2 = np.asarray(bias2, dtype=np.float32)
    wg = np.asarray(weights_g, dtype=np.float32)
    bgv = np.asarray(bias_g, dtype=np.float32)

    if np.any(b1 != 0.0):
        # hinge-folding below needs b1 == 0; exact general fallback
        return _numpy_fallback(x, w1, b1, w2, b2, wg, bgv)

    # fold the E=4 expand/shrink into two per-row coefficients
    c = (w2 * np.abs(w1)).sum(axis=1)           # [R]
    d = (w2 * np.minimum(w1, 0.0)).sum(axis=1)  # [R]
    G = N_GENES

    # per-gene scalar table [G_PAD, NCOL]: wg0, wg1, bg, b20, b21
    wtab = np.zeros((G_PAD, NCOL), dtype=np.float32)
    wtab[:G, 0] = wg[:, 0]
    wtab[:G, 1] = wg[:, 1]
    wtab[:G, 2] = bgv
    wtab[:G, 3] = b2[:G]
    wtab[:G, 4] = b2[G:]

    # per-gene diag coefficients [G_PAD, 4]: c0, d0, c1, d1
    ctab = np.zeros((G_PAD, 4), dtype=np.float32)
    ctab[:G, 0] = c[:G]
    ctab[:G, 1] = d[:G]
    ctab[:G, 2] = c[G:]
    ctab[:G, 3] = d[G:]

    # x -> [G_PAD, T, B] fp16, contiguous per gene row
    xt = np.zeros((G_PAD, N_TECH, BATCH), dtype=np.float16)
    xt[:G] = x.transpose(2, 1, 0)

    idx = np.arange(P)
    in_maps = []
    for i in range(N_CORES):
        g0 = i * GS
        xi = np.ascontiguousarray(xt[g0 : g0 + GS].reshape(NTILES, P, 2 * FD))
        wi = np.ascontiguousarray(
            wtab[g0 : g0 + GS].reshape(NTILES, P, NCOL).transpose(1, 0, 2)
            .reshape(P, NTILES * NCOL)
        )
        # diagonal stationaries [NTILES, 4, P(k), P(m)] -> [P, NTILES*4*P]
        ci = ctab[g0 : g0 + GS].reshape(NTILES, P, 4)
        dgi = np.zeros((NTILES, 4, P, P), dtype=np.float16)
        for k in range(4):
            dgi[:, k, idx, idx] = ci[:, :, k]
        dgi = np.ascontiguousarray(
            dgi.transpose(2, 0, 1, 3).reshape(P, NTILES * 4 * P)
        )
        in_maps.append({"x": xi, "w": wi, "dg": dgi})

    has_b2 = bool(np.any(b2 != 0.0))
    if has_b2 not in _nc_cache:
        _nc_cache[has_b2] = _build_nc(has_b2)
    nc = _nc_cache[has_b2]

    from concourse.bass_utils import run_bass_kernel_spmd

    trace = bool(int(os.environ.get("KERNEL_TRACE", "0")))
    res = run_bass_kernel_spmd(nc, in_maps, core_ids=list(range(N_CORES)),
                               trace=trace)
    LAST_EXEC_NS = res.exec_time_ns
    LAST_RESULTS = res

    parts = [res.results[i]["out"].reshape(GS, BATCH) for i in range(N_CORES)]
    full = np.concatenate(parts, axis=0)[:G]          # [G, B] fp16
    return np.ascontiguousarray(full.T).astype(np.float32)



# revision 5
# speedup vs baseline: 1.3295x; 1.3295x over previous
"""Trainium2 SPMD kernel for nn_CombinedGeneModel (v2: sign-bucketed).

Math (per batch b, tech t, gene g; R = T*G independent tiny MLPs):
    h   = relu(x * w1[r,e] + b1[r,e])          e = 0..3
    s   = relu(sum_e h*w2[r,e] + b2[r])
    out = relu(sum_t s[b,t,g]*wg[g,t] + bg[g])

With b1 == 0 (guaranteed by setup_inputs) the E=4 hinge sum folds exactly:
    sum_e w2_e*relu(w1_e*x) = c*relu(x) + d*x
      c = sum_e w2_e*|w1_e|,  d = sum_e w2_e*min(w1_e, 0)
so per row:  s = relu(c*relu(x) + d*x + b2).

v2 restructure: fold |wg_t| into the coefficients (c' = |wg|c, d' = |wg|d,
b2' = |wg|b2) so the mid relu directly yields v_t = |wg_t|*s_t >= 0, and
    out = relu(sign(wg0)*v0 + sign(wg1)*v1 + bg).
Genes sign-bucketed on the host:
  A (+,+): out = relu(v0 + v1 + bg)        -> ADD tiles
  B mixed: swap techs so slot0 is the + one -> SUB tiles, relu(v0 - v1 + bg)
  C (-,-) & bg<=0: out == 0 exactly         -> dropped, no DMA/compute
  D (-,-) & bg>0:  out = relu(bg - v0 - v1) -> RSUB tiles (empty in practice)
~25% of genes fall in C: that work (DMA + all engines) disappears.

Layout: genes on SBUF partitions, batch on the free axis; tiles sharded
across the 8 NeuronCores; host pre-transposes x to [tile, P, T*B] fp16.

Engine split per 128-gene tile (free dim 2*1024):
  VectorE: p = relu(x)  ([128,2048] fp16, 2x rate)
  TensorE: u_t = diag(c'_t) @ p_t + diag(d'_t) @ x_t -> PSUM f32, 8 matmuls
           (+ a few junk warm-up matmuls up front to flip the HAM clock gate)
  ScalarE: v = relu(u)  one [128,2048] op PSUM -> SBUF fp16 (4 psum banks)
  VectorE: tmp = v0 +/- v1 ; out = max(tmp + bg, 0) -> output staging
  Sync   : HWDGE x loads + per-tile output stores, interleaved

The walrus build accepts at most ONE sync wait per instruction, so the
kernel keeps the baseline's tricks: no-reuse buffers for DMA targets, tiny
same-engine "touch"/dummy-ldweights ops to absorb extra semaphore waits,
and a post-pass that splits any remaining multi-wait instruction.
"""

import os
import numpy as np

N_GENES = 20000
N_TECH = 2
BATCH = 1024
N_CORES = 8
P = 128
FD = BATCH               # free dim per (tile, tech)
HF = FD // 2             # matmul moving-free-dim limit is 512
NCOL = 3                 # per-gene scalars: bg, b2'0, b2'1
N_WARM = 6               # junk matmuls to pre-warm the PE HAM clock gate

LAST_EXEC_NS = None
LAST_RESULTS = None

_nc_cache = {}


def _build_nc(kinds: tuple, has_bias: bool):
    """kinds: per-tile op kind, 'A' (v0+v1), 'B' (v0-v1), 'D' (bg-v0-v1)."""
    import concourse.bass as bass
    import concourse.mybir as mybir
    from concourse.tile import TileContext

    Op = mybir.AluOpType
    Act = mybir.ActivationFunctionType
    f16 = mybir.dt.float16
    f32 = mybir.dt.float32

    NT = len(kinds)
    # store-chunk boundaries: <=8 stores, sized evenly (larger chunks first,
    # so the final store — the latency tail — is the small one)
    n_st = min(8, NT)
    base, extra = divmod(NT, n_st)
    sizes = [base + (1 if i < extra else 0) for i in range(n_st)]
    store_after, store_start = set(), {}
    pos = 0
    for sz in sizes:
        store_after.add(pos + sz - 1)
        store_start[pos + sz - 1] = pos
        pos += sz

    nc = bass.Bass()
    x_d = nc.declare_dram_parameter("x", [NT, P, 2 * FD], f16, isOutput=False)
    w_d = nc.declare_dram_parameter("w", [P, NT * NCOL], f32, isOutput=False)
    g_d = nc.declare_dram_parameter("dg", [P, NT * 4 * P], f16, isOutput=False)
    o_d = nc.declare_dram_parameter("out", [NT, P, FD], f16, isOutput=True)

    with TileContext(nc) as tc:
        with (
            tc.tile_pool(name="wp", bufs=1) as wpool,
            tc.tile_pool(name="xp", bufs=NT) as xpool,
            tc.tile_pool(name="op", bufs=1) as opool,
            tc.tile_pool(name="pp", bufs=4) as ppool,
            tc.tile_pool(name="sp", bufs=4) as spool,
            tc.tile_pool(name="tp", bufs=4) as tpool,
            tc.tile_pool(name="sc", bufs=NT) as scpool,
            tc.tile_pool(name="ps", bufs=2, space="PSUM") as pspool,
        ):
            obuf = opool.tile([P, NT * FD], f16)
            w = wpool.tile([P, NT * NCOL], f32)
            nc.sync.dma_start(w[:], w_d[:])
            # junk tile for PE warm-up matmuls (no DMA dependency)
            junk = wpool.tile([P, HF], f16)
            nc.vector.memset(junk[:], 0.0)
            # diag load split: a small head chunk lands before the x loads
            # so the first tiles' matmuls are not gated on the full load
            dg = wpool.tile([P, NT * 4 * P], f16)
            DG_HEAD = min(3, NT) * 4 * P
            nc.sync.dma_start(dg[:, 0:DG_HEAD], g_d[:, 0:DG_HEAD])
            if NT > 3:
                nc.sync.dma_start(dg[:, DG_HEAD:], g_d[:, DG_HEAD:])

            # absorb the w/dg DMA waits once per consuming engine
            wt_v = wpool.tile([P, 1], f32)
            nc.vector.tensor_copy(wt_v[:], w[:, 0:1])
            wt_a = wpool.tile([P, 1], f32)
            nc.scalar.copy(wt_a[:], w[:, 0:1])
            nc.tensor.ldweights(dg[:, 0:P])       # PE touch: head diag chunk
            if NT > 3:
                nc.tensor.ldweights(dg[:, DG_HEAD : DG_HEAD + P])

            # PE HAM warm-up: junk matmuls during the first x-tile DMA so the
            # clock gate flips to 2.4 GHz before real matmuls arrive.  tile 0
            # re-clears the bank via start=True.
            pwarm = pspool.tile([P, 2 * FD], f32, tag="u")
            for i in range(N_WARM):
                nc.tensor.matmul(pwarm[:, 0:HF], junk[:, 0:P], junk[:],
                                 start=(i == 0), stop=(i == N_WARM - 1))

            xts = []
            sbs = []
            for j in range(NT):
                kind = kinds[j]
                col = j * NCOL
                bg = w[:, col + 0 : col + 1]
                bs0 = w[:, col + 1 : col + 2]
                bs1 = w[:, col + 2 : col + 3]
                dgc = j * 4 * P
                dg_c0 = dg[:, dgc + 0 * P : dgc + 1 * P]
                dg_d0 = dg[:, dgc + 1 * P : dgc + 2 * P]
                dg_c1 = dg[:, dgc + 2 * P : dgc + 3 * P]
                dg_d1 = dg[:, dgc + 3 * P : dgc + 4 * P]

                xt = xpool.tile([P, 2 * FD], f16, tag="x")
                nc.sync.dma_start(xt[:], x_d[j])
                xts.append(xt)

                # DVE: absorb this load's DMA-lane wait, then p = relu(x)
                scx = scpool.tile([P, 1], f16, tag="scx")
                nc.vector.tensor_copy(scx[:], xt[:, 0:1])
                pf = ppool.tile([P, 2 * FD], f16, tag="p")
                nc.vector.tensor_scalar(pf[:], xt[:], 0.0, None, Op.max)

                # TensorE: u_t = diag(c'_t)@p_t + diag(d'_t)@x_t in PSUM.
                # dummy load_weights absorbs the DVE(p-ready) wait.
                nc.tensor.ldweights(pf[:, 0:P])
                uh = pspool.tile([P, 2 * FD], f32, tag="u")
                for t in range(2):
                    dc = dg_c0 if t == 0 else dg_c1
                    dd = dg_d0 if t == 0 else dg_d1
                    for h in range(2):
                        s = slice(t * FD + h * HF, t * FD + (h + 1) * HF)
                        nc.tensor.matmul(uh[:, s], dc, pf[:, s],
                                         start=True, stop=False)
                        nc.tensor.matmul(uh[:, s], dd, xt[:, s],
                                         start=False, stop=True)

                # ScalarE: v = relu(u), PSUM -> SBUF fp16, one [128,2048] op.
                # scribble on the outgoing v-tile first: the tiny write
                # carries the WAR wait (DVE readers of sb[j-4]) and its WAW
                # ordering pins it before the relu that reuses the slot.
                if j >= 4:
                    nc.scalar.copy(sbs[j - 4][:, 0:1], wt_a[:])
                sb = spool.tile([P, 2 * FD], f16, tag="s")
                sbs.append(sb)
                if has_bias:
                    nc.scalar.activation(sb[:, 0:FD], uh[:, 0:FD],
                                         Act.Relu, bias=bs0)
                    nc.scalar.activation(sb[:, FD : 2 * FD], uh[:, FD : 2 * FD],
                                         Act.Relu, bias=bs1)
                else:
                    nc.scalar.activation(sb[:], uh[:], Act.Relu)

                # VectorE: combine + final relu
                v0 = sb[:, 0:FD]
                v1 = sb[:, FD : 2 * FD]
                ot = obuf[:, j * FD : (j + 1) * FD]
                tmp = tpool.tile([P, FD], f16, tag="t")
                if kind == "A":
                    nc.vector.tensor_tensor(tmp[:], v0, v1, Op.add)
                    nc.vector.tensor_scalar(ot, tmp[:], bg, 0.0, Op.add, Op.max)
                elif kind == "B":
                    nc.vector.tensor_tensor(tmp[:], v0, v1, Op.subtract)
                    nc.vector.tensor_scalar(ot, tmp[:], bg, 0.0, Op.add, Op.max)
                else:  # D: relu(bg - v0 - v1)
                    nc.vector.tensor_tensor(tmp[:], v0, v1, Op.add)
                    t2 = tpool.tile([P, FD], f16, tag="t2")
                    nc.vector.tensor_scalar(t2[:], tmp[:], -1.0, bg,
                                            Op.mult, Op.add)
                    nc.vector.tensor_scalar(ot, t2[:], 0.0, None, Op.max)

                # chunked SWDGE stores: at most 8, so each lands on a virgin
                # SWDGE lane and carries only the DVE data wait (walrus
                # accepts at most one sync wait per instruction)
                if j in store_after:
                    k0 = store_start[j]
                    src = obuf[:, k0 * FD : (j + 1) * FD].rearrange(
                        "p (t b) -> p t b", t=j + 1 - k0
                    )
                    dst = o_d[k0 : j + 1].rearrange("t p b -> p t b")
                    nc.gpsimd.dma_start(dst, src)

    _split_multi_waits(nc, mybir)
    return nc


def _split_multi_waits(nc, mybir):
    """walrus (gen3 codegen here) accepts at most one sync wait per
    instruction.  Two rewrites keep every instruction at <=1 wait:

    1. Drop self-engine waits that are provably satisfied: engines run
       their stream in order and bump their own semaphore once per
       retired instruction, so a wait on the engine's own semaphore for
       a value already reached earlier in its own stream is a no-op
       (Tile emits these because its clock tracking is not transitive).
    2. For the remaining multi-wait instructions (the epilogue Drain,
       which is block-initial), hoist all but one wait onto same-engine
       NoOps appended to the preceding basic block."""
    blocks = list(nc.main_func.blocks)

    # sem id -> set of engines that increment it
    updaters = {}
    for bb in blocks:
        for ins in bb.instructions:
            si = getattr(ins, "sync_info", None)
            if si is None:
                continue
            for up in si.on_update or []:
                updaters.setdefault(up.id, set()).add(ins.engine)

    # pass 1: strip satisfied self-waits, walking in block order while
    # accumulating each semaphore's increments
    cum = {}
    for bb in blocks:
        for ins in bb.instructions:
            si = getattr(ins, "sync_info", None)
            if si is None:
                continue
            waits = list(si.on_wait or [])
            if len(waits) > 1:
                kept = []
                for wv in waits:
                    if (
                        wv.sync_type == "semaphore"
                        and wv.wait_mode == "sem-ge-imm"
                        and updaters.get(wv.id) == {ins.engine}
                        # engine sems increment at in-order instruction
                        # retirement, so earlier-stream increments prove the
                        # wait satisfied; DMA lane sems (DMAHW*/DMASW*)
                        # increment at async DMA *completion* — never strip
                        and "DMA" not in (wv.ant_name or "")
                        and cum.get(wv.id, 0) >= wv.wait_value
                    ):
                        continue  # provably satisfied self-wait
                    kept.append(wv)
                if len(kept) != len(waits):
                    ins.sync_info = mybir.SyncInfo(
                        on_wait=kept, on_update=list(si.on_update or [])
                    )
            si = ins.sync_info
            for up in si.on_update or []:
                if up.update_mode == "sem-inc":
                    cum[up.id] = cum.get(up.id, 0) + up.update_value

    # pass 2: NoOp-split anything still multi-wait (the Drain)
    nop_idx = 0
    for bi, bb in enumerate(blocks):
        for ins in bb.instructions:
            si = getattr(ins, "sync_info", None)
            if si is None:
                continue
            waits = list(si.on_wait or [])
            if len(waits) <= 1:
                continue
            assert bi > 0, f"multi-wait instruction in first block: {ins.name}"
            for other in bb.instructions:
                if other.name == ins.name:
                    break
                assert other.engine != ins.engine, (
                    f"cannot NoOp-split mid-block instruction {ins.name}"
                )
            prev_bb = blocks[bi - 1]
            for wv in waits[:-1]:
                nop = mybir.InstNoOp(name=f"ant-waitsplit-{nop_idx}")
                nop_idx += 1
                nop.engine = ins.engine
                nop.sync_info = mybir.SyncInfo(on_wait=[wv], on_update=[])
                prev_bb.add_instruction(nop)
            ins.sync_info = mybir.SyncInfo(
                on_wait=[waits[-1]], on_update=list(si.on_update or [])
            )


def _numpy_fallback(x, w1, b1, w2, b2, wg, bgv):
    B = x.shape[0]
    R = N_GENES * N_TECH
    xr = x.reshape(B, R).T.astype(np.float32)
    h = np.maximum(xr[:, :, None] * w1[:, None, :] + b1[:, None, :], 0.0)
    s = np.maximum(np.einsum("rbe,re->rb", h, w2) + b2[:, None], 0.0)
    s = s.T.reshape(B, N_TECH, N_GENES)
    out = np.maximum(np.einsum("btg,gt->bg", s, wg) + bgv, 0.0)
    return out.astype(np.float32)


def kernel(x, weights1, bias1, weights2, bias2, weights_g, bias_g):
    global LAST_EXEC_NS, LAST_RESULTS
    x = np.asarray(x, dtype=np.float32)
    w1 = np.asarray(weights1, dtype=np.float32)
    b1 = np.asarray(bias1, dtype=np.float32)
    w2 = np.asarray(weights2, dtype=np.float32)
    b2 = np.asarray(bias2, dtype=np.float32)
    wg = np.asarray(weights_g, dtype=np.float32)
    bgv = np.asarray(bias_g, dtype=np.float32)

    if np.any(b1 != 0.0):
        # hinge-folding below needs b1 == 0; exact general fallback
        return _numpy_fallback(x, w1, b1, w2, b2, wg, bgv)

    G = N_GENES
    # fold the E=4 expand/shrink into two per-row coefficients
    c = (w2 * np.abs(w1)).sum(axis=1)           # [R]
    d = (w2 * np.minimum(w1, 0.0)).sum(axis=1)  # [R]
    ct = np.stack([c[:G], c[G:]], axis=1)       # [G, T]
    dt_ = np.stack([d[:G], d[G:]], axis=1)
    b2t = np.stack([b2[:G], b2[G:]], axis=1)

    awg = np.abs(wg)
    # fold |wg| so the mid relu yields v_t = |wg_t| * s_t
    cp = awg * ct                                # [G, T]
    dp = awg * dt_
    bp = awg * b2t

    pos = wg >= 0.0
    both_pos = pos[:, 0] & pos[:, 1]
    both_neg = (~pos[:, 0]) & (~pos[:, 1])
    mixed = ~(both_pos | both_neg)
    gA = np.nonzero(both_pos)[0]
    gB = np.nonzero(mixed)[0]
    gC = np.nonzero(both_neg & (bgv <= 0.0))[0]
    gD = np.nonzero(both_neg & (bgv > 0.0))[0]

    # per-bucket global tile counts, padded to a multiple of N_CORES
    def n_tiles(n):
        t = -(-n // P)
        return -(-t // N_CORES) * N_CORES if n else 0

    TA, TB, TD = n_tiles(len(gA)), n_tiles(len(gB)), n_tiles(len(gD))
    kinds_pc = ("A",) * (TA // N_CORES) + ("B",) * (TB // N_CORES) + (
        "D",) * (TD // N_CORES)
    NT = len(kinds_pc)

    # global slot table: per bucket, genes padded with -1 to TA*P slots,
    # then chunked per core (core i takes tiles [i*TA/8, (i+1)*TA/8) etc.)
    def pad_slots(g, T):
        s = np.full(T * P, -1, dtype=np.int64)
        s[: len(g)] = g
        return s.reshape(N_CORES, -1) if T else s.reshape(N_CORES, 0)

    slots = np.concatenate(
        [pad_slots(gA, TA), pad_slots(gB, TB), pad_slots(gD, TD)], axis=1
    )  # [N_CORES, NT*P]

    # tech order: slot0 must hold the + tech (identity except mixed genes
    # whose tech0 weight is negative)
    t0 = np.where(mixed & ~pos[:, 0], 1, 0)     # [G]
    t1 = 1 - t0
    tord = np.stack([t0, t1], axis=1)           # [G, T]

    # x -> [G, T, B] fp16 view for gathering
    xt_full = np.ascontiguousarray(x.transpose(2, 1, 0))  # [G, T, B] f32

    idx = np.arange(P)
    in_maps = []
    for i in range(N_CORES):
        gs = slots[i]                            # [NT*P]
        valid = gs >= 0
        gsafe = np.where(valid, gs, 0)
        to = tord[gsafe]                         # [NT*P, T]
        to[~valid] = 0

        xi = np.take_along_axis(
            xt_full[gsafe], to[:, :, None], axis=1
        )                                        # [NT*P, T, B]
        xi[~valid] = 0.0
        xi = xi.reshape(NT, P, 2 * FD).astype(np.float16)

        cpi = np.take_along_axis(cp[gsafe], to, axis=1)   # [NT*P, T]
        dpi = np.take_along_axis(dp[gsafe], to, axis=1)
        bpi = np.take_along_axis(bp[gsafe], to, axis=1)
        bgi = bgv[gsafe].copy()
        for a in (cpi, dpi, bpi):
            a[~valid] = 0.0
        bgi[~valid] = 0.0

        wi = np.zeros((NT, P, NCOL), dtype=np.float32)
        wi[:, :, 0] = bgi.reshape(NT, P)
        wi[:, :, 1] = bpi[:, 0].reshape(NT, P)
        wi[:, :, 2] = bpi[:, 1].reshape(NT, P)
        wi = np.ascontiguousarray(
            wi.transpose(1, 0, 2).reshape(P, NT * NCOL)
        )

        # diagonal stationaries [NT, 4, P(k), P(m)] -> [P, NT*4*P]
        coef = np.stack(
            [cpi[:, 0], dpi[:, 0], cpi[:, 1], dpi[:, 1]], axis=1
        ).reshape(NT, P, 4)
        dgi = np.zeros((NT, 4, P, P), dtype=np.float16)
        for k in range(4):
            dgi[:, k, idx, idx] = coef[:, :, k]
        dgi = np.ascontiguousarray(
            dgi.transpose(2, 0, 1, 3).reshape(P, NT * 4 * P)
        )
        in_maps.append({"x": xi, "w": wi, "dg": dgi})

    has_bias = bool(np.any(bp != 0.0))

    if os.environ.get("KERNEL_NUMPY"):
        # emulate the device dataflow (fp16 rounding included) to validate
        # the host prep without hardware
        results = []
        for i in range(N_CORES):
            xi = in_maps[i]["x"].astype(np.float32).reshape(NT * P, 2, FD)
            wi = in_maps[i]["w"].reshape(P, NT, NCOL).transpose(1, 0, 2)
            dgi = (
                in_maps[i]["dg"].astype(np.float32)
                .reshape(P, NT, 4, P).transpose(1, 2, 0, 3)
            )  # [NT, 4, Pk, Pm]
            coef = dgi[:, :, idx, idx].transpose(0, 2, 1)  # [NT, P, 4]
            coef = coef.reshape(NT * P, 4)
            pfi = np.maximum(xi, 0.0).astype(np.float16).astype(np.float32)
            u0 = coef[:, 0:1] * pfi[:, 0] + coef[:, 1:2] * xi[:, 0]
            u1 = coef[:, 2:3] * pfi[:, 1] + coef[:, 3:4] * xi[:, 1]
            bsl = wi[:, :, 1:3].reshape(NT * P, 2)
            v0 = np.maximum(u0 + bsl[:, 0:1], 0.0).astype(np.float16).astype(np.float32)
            v1 = np.maximum(u1 + bsl[:, 1:2], 0.0).astype(np.float16).astype(np.float32)
            bgl = wi[:, :, 0].reshape(NT * P, 1)
            o = np.zeros_like(v0)
            for jt, kd in enumerate(kinds_pc):
                sl = slice(jt * P, (jt + 1) * P)
                if kd == "A":
                    t = v0[sl] + v1[sl]
                elif kd == "B":
                    t = v0[sl] - v1[sl]
                else:
                    t = -v0[sl] - v1[sl]
                o[sl] = np.maximum(t + bgl[sl], 0.0)
            results.append({"out": o.astype(np.float16)})

        class _R:
            pass

        res = _R()
        res.results = results
        res.exec_time_ns = None
    else:
        key = (kinds_pc, has_bias)
        if key not in _nc_cache:
            _nc_cache[key] = _build_nc(kinds_pc, has_bias)
        nc = _nc_cache[key]

        from concourse.bass_utils import run_bass_kernel_spmd

        trace = bool(int(os.environ.get("KERNEL_TRACE", "0")))
        res = run_bass_kernel_spmd(nc, in_maps, core_ids=list(range(N_CORES)),
                                   trace=trace)
    LAST_EXEC_NS = res.exec_time_ns
    LAST_RESULTS = res

    # assemble: [G, B] then transpose; C-bucket genes and padding stay 0
    outT = np.zeros((G, BATCH), dtype=np.float32)
    for i in range(N_CORES):
        flat = np.asarray(res.results[i]["out"]).reshape(NT * P, BATCH)
        gs = slots[i]
        valid = gs >= 0
        outT[gs[valid]] = flat[valid]
    return np.ascontiguousarray(outT.T)


# revision 8
# speedup vs baseline: 1.4083x; 1.0593x over previous
"""Trainium2 SPMD kernel for nn_CombinedGeneModel (v2: sign-bucketed).

Math (per batch b, tech t, gene g; R = T*G independent tiny MLPs):
    h   = relu(x * w1[r,e] + b1[r,e])          e = 0..3
    s   = relu(sum_e h*w2[r,e] + b2[r])
    out = relu(sum_t s[b,t,g]*wg[g,t] + bg[g])

With b1 == 0 (guaranteed by setup_inputs) the E=4 hinge sum folds exactly:
    sum_e w2_e*relu(w1_e*x) = c*relu(x) + d*x
      c = sum_e w2_e*|w1_e|,  d = sum_e w2_e*min(w1_e, 0)
so per row:  s = relu(c*relu(x) + d*x + b2).

v2 restructure: fold |wg_t| into the coefficients (c' = |wg|c, d' = |wg|d,
b2' = |wg|b2) so the mid relu directly yields v_t = |wg_t|*s_t >= 0, and
    out = relu(sign(wg0)*v0 + sign(wg1)*v1 + bg).
Genes sign-bucketed on the host:
  A (+,+): out = relu(v0 + v1 + bg)        -> ADD tiles
  B mixed: swap techs so slot0 is the + one -> SUB tiles, relu(v0 - v1 + bg)
  C (-,-) & bg<=0: out == 0 exactly         -> dropped, no DMA/compute
  D (-,-) & bg>0:  out = relu(bg - v0 - v1) -> RSUB tiles (empty in practice)
~25% of genes fall in C: that work (DMA + all engines) disappears.

Layout: genes on SBUF partitions, batch on the free axis; tiles sharded
across the 8 NeuronCores; host pre-transposes x to [tile, P, T*B] fp16.

Engine split per 128-gene tile (free dim 2*1024):
  VectorE: p = relu(x)  ([128,2048] fp16, 2x rate)
  TensorE: u_t = diag(c'_t) @ p_t + diag(d'_t) @ x_t -> PSUM f32, 8 matmuls
           (+ a few junk warm-up matmuls up front to flip the HAM clock gate)
  ScalarE: v = relu(u)  one [128,2048] op PSUM -> SBUF fp16 (4 psum banks)
  VectorE: tmp = v0 +/- v1 ; out = max(tmp + bg, 0) -> output staging
  Sync   : HWDGE x loads + per-tile output stores, interleaved

The walrus build accepts at most ONE sync wait per instruction, so the
kernel keeps the baseline's tricks: no-reuse buffers for DMA targets, tiny
same-engine "touch"/dummy-ldweights ops to absorb extra semaphore waits,
and a post-pass that splits any remaining multi-wait instruction.
"""

import os
import numpy as np

N_GENES = 20000
N_TECH = 2
BATCH = 1024
N_CORES = 8
P = 128
FD = BATCH               # free dim per (tile, tech)
HF = FD // 2             # matmul moving-free-dim limit is 512
NCOL = 3                 # per-gene scalars: bg, b2'0, b2'1
N_WARM = 10              # junk matmuls to pre-warm the PE HAM clock gate

LAST_EXEC_NS = None
LAST_RESULTS = None

_nc_cache = {}


def _build_nc(kinds: tuple, has_bias: bool):
    """kinds: per-tile op kind, 'A' (v0+v1), 'B' (v0-v1), 'D' (bg-v0-v1)."""
    import concourse.bass as bass
    import concourse.mybir as mybir
    from concourse.tile import TileContext

    Op = mybir.AluOpType
    Act = mybir.ActivationFunctionType
    f16 = mybir.dt.float16
    f32 = mybir.dt.float32

    NT = len(kinds)
    # store-chunk boundaries: <=8 stores, sized evenly (larger chunks first,
    # so the final store — the latency tail — is the small one)
    n_st = min(8, NT)
    base, extra = divmod(NT, n_st)
    sizes = [base + (1 if i < extra else 0) for i in range(n_st)]
    store_after, store_start = set(), {}
    pos = 0
    for sz in sizes:
        store_after.add(pos + sz - 1)
        store_start[pos + sz - 1] = pos
        pos += sz

    nc = bass.Bass()
    x_d = nc.declare_dram_parameter("x", [NT, P, 2 * FD], f16, isOutput=False)
    w_d = nc.declare_dram_parameter("w", [P, NT * NCOL], f32, isOutput=False)
    g_d = nc.declare_dram_parameter("dg", [P, NT * 4 * P], f16, isOutput=False)
    o_d = nc.declare_dram_parameter("out", [NT, P, FD], f16, isOutput=True)

    with TileContext(nc) as tc:
        with (
            tc.tile_pool(name="wp", bufs=1) as wpool,
            tc.tile_pool(name="xp", bufs=NT) as xpool,
            tc.tile_pool(name="op", bufs=1) as opool,
            tc.tile_pool(name="pp", bufs=4) as ppool,
            tc.tile_pool(name="sp", bufs=4) as spool,
            tc.tile_pool(name="tp", bufs=4) as tpool,
            tc.tile_pool(name="sc", bufs=NT) as scpool,
            tc.tile_pool(name="gp", bufs=NT) as gpool,
            tc.tile_pool(name="ps", bufs=2, space="PSUM") as pspool,
        ):
            obuf = opool.tile([P, NT * FD], f16)
            w = wpool.tile([P, NT * NCOL], f32)
            nc.sync.dma_start(w[:], w_d[:])
            # junk tile for PE warm-up matmuls (no DMA dependency)
            junk = wpool.tile([P, P], f16)
            nc.vector.memset(junk[:], 0.0)

            # per-tile loads, interleaved on the sync HWDGE FIFO ring so
            # tile j's x and diag chunks complete at ~j*1.8us — well ahead
            # of the ~3us/tile compute cadence (all DMA targets no-reuse)
            xts, dgs = [], []
            for j in range(NT):
                xt = xpool.tile([P, 2 * FD], f16, tag="x")
                nc.sync.dma_start(xt[:], x_d[j])
                xts.append(xt)
                dgt = gpool.tile([P, 4 * P], f16, tag="dg")
                nc.sync.dma_start(dgt[:], g_d[:, j * 4 * P : (j + 1) * 4 * P])
                dgs.append(dgt)

            # absorb the w DMA waits once per consuming engine
            wt_v = wpool.tile([P, 1], f32)
            nc.vector.tensor_copy(wt_v[:], w[:, 0:1])
            wt_a = wpool.tile([P, 1], f32)
            nc.scalar.copy(wt_a[:], w[:, 0:1])

            # PE HAM warm-up: junk matmuls during the first x-tile DMA so the
            # clock gate flips to 2.4 GHz soon after real matmuls arrive.
            # tile 0 re-clears the bank via start=True.
            pwarm = pspool.tile([P, 2 * FD], f32, tag="u")
            for i in range(N_WARM):
                nc.tensor.matmul(pwarm[:, 0:P], junk[:], junk[:],
                                 start=(i == 0), stop=(i == N_WARM - 1))

            sbs = []
            for j in range(NT):
                kind = kinds[j]
                col = j * NCOL
                bg = w[:, col + 0 : col + 1]
                bs0 = w[:, col + 1 : col + 2]
                bs1 = w[:, col + 2 : col + 3]
                dgt = dgs[j]
                xt = xts[j]

                # DVE: absorb this load's DMA-lane wait, then p = relu(x)
                scx = scpool.tile([P, 1], f16, tag="scx")
                nc.vector.tensor_copy(scx[:], xt[:, 0:1])
                pf = ppool.tile([P, 2 * FD], f16, tag="p")
                nc.vector.tensor_scalar(pf[:], xt[:], 0.0, None, Op.max)

                # TensorE: u_t = diag(c'_t)@p_t + diag(d'_t)@x_t in PSUM.
                # Dummy 1-col ldweights absorb the DVE(p-ready) and this
                # tile's diag-DMA waits.  Matmuls are ordered so each
                # stationary is loaded once and used by two back-to-back
                # matmuls (both batch halves), which pipeline at ~N cycles
                # instead of paying the isolated fill+drain cost.
                nc.tensor.ldweights(pf[:, 0:1])
                nc.tensor.ldweights(dgt[:, 0:1])
                uh = pspool.tile([P, 2 * FD], f32, tag="u")
                for t in range(2):
                    dc = dgt[:, (2 * t) * P : (2 * t + 1) * P]
                    dd = dgt[:, (2 * t + 1) * P : (2 * t + 2) * P]
                    for h in range(2):
                        s = slice(t * FD + h * HF, t * FD + (h + 1) * HF)
                        nc.tensor.matmul(uh[:, s], dc, pf[:, s],
                                         start=True, stop=False)
                    for h in range(2):
                        s = slice(t * FD + h * HF, t * FD + (h + 1) * HF)
                        nc.tensor.matmul(uh[:, s], dd, xt[:, s],
                                         start=False, stop=True)

                # ScalarE: v = relu(u), PSUM -> SBUF fp16, one [128,2048] op.
                # scribble on the outgoing v-tile first: the tiny write
                # carries the WAR wait (DVE readers of sb[j-4]) and its WAW
                # ordering pins it before the relu that reuses the slot.
                if j >= 4:
                    nc.scalar.copy(sbs[j - 4][:, 0:1], wt_a[:])
                sb = spool.tile([P, 2 * FD], f16, tag="s")
                sbs.append(sb)
                if has_bias:
                    nc.scalar.activation(sb[:, 0:FD], uh[:, 0:FD],
                                         Act.Relu, bias=bs0)
                    nc.scalar.activation(sb[:, FD : 2 * FD], uh[:, FD : 2 * FD],
                                         Act.Relu, bias=bs1)
                else:
                    nc.scalar.activation(sb[:], uh[:], Act.Relu)

                # VectorE: combine + final relu
                v0 = sb[:, 0:FD]
                v1 = sb[:, FD : 2 * FD]
                ot = obuf[:, j * FD : (j + 1) * FD]
                tmp = tpool.tile([P, FD], f16, tag="t")
                if kind == "A":
                    nc.vector.tensor_tensor(tmp[:], v0, v1, Op.add)
                    nc.vector.tensor_scalar(ot, tmp[:], bg, 0.0, Op.add, Op.max)
                elif kind == "B":
                    nc.vector.tensor_tensor(tmp[:], v0, v1, Op.subtract)
                    nc.vector.tensor_scalar(ot, tmp[:], bg, 0.0, Op.add, Op.max)
                else:  # D: relu(bg - v0 - v1)
                    nc.vector.tensor_tensor(tmp[:], v0, v1, Op.add)
                    t2 = tpool.tile([P, FD], f16, tag="t2")
                    nc.vector.tensor_scalar(t2[:], tmp[:], -1.0, bg,
                                            Op.mult, Op.add)
                    nc.vector.tensor_scalar(ot, t2[:], 0.0, None, Op.max)

                # chunked SWDGE stores: at most 8, so each lands on a virgin
                # SWDGE lane and carries only the DVE data wait (walrus
                # accepts at most one sync wait per instruction)
                if j in store_after:
                    k0 = store_start[j]
                    src = obuf[:, k0 * FD : (j + 1) * FD].rearrange(
                        "p (t b) -> p t b", t=j + 1 - k0
                    )
                    dst = o_d[k0 : j + 1].rearrange("t p b -> p t b")
                    nc.gpsimd.dma_start(dst, src)

    _split_multi_waits(nc, mybir)
    return nc


def _split_multi_waits(nc, mybir):
    """walrus (gen3 codegen here) accepts at most one sync wait per
    instruction.  Two rewrites keep every instruction at <=1 wait:

    1. Drop self-engine waits that are provably satisfied: engines run
       their stream in order and bump their own semaphore once per
       retired instruction, so a wait on the engine's own semaphore for
       a value already reached earlier in its own stream is a no-op
       (Tile emits these because its clock tracking is not transitive).
    2. For the remaining multi-wait instructions (the epilogue Drain,
       which is block-initial), hoist all but one wait onto same-engine
       NoOps appended to the preceding basic block."""
    blocks = list(nc.main_func.blocks)

    # sem id -> set of engines that increment it
    updaters = {}
    for bb in blocks:
        for ins in bb.instructions:
            si = getattr(ins, "sync_info", None)
            if si is None:
                continue
            for up in si.on_update or []:
                updaters.setdefault(up.id, set()).add(ins.engine)

    # pass 1: strip satisfied self-waits, walking in block order while
    # accumulating each semaphore's increments
    cum = {}
    for bb in blocks:
        for ins in bb.instructions:
            si = getattr(ins, "sync_info", None)
            if si is None:
                continue
            waits = list(si.on_wait or [])
            if len(waits) > 1:
                kept = []
                for wv in waits:
                    if (
                        wv.sync_type == "semaphore"
                        and wv.wait_mode == "sem-ge-imm"
                        and updaters.get(wv.id) == {ins.engine}
                        # engine sems increment at in-order instruction
                        # retirement, so earlier-stream increments prove the
                        # wait satisfied; DMA lane sems (DMAHW*/DMASW*)
                        # increment at async DMA *completion* — never strip
                        and "DMA" not in (wv.ant_name or "")
                        and cum.get(wv.id, 0) >= wv.wait_value
                    ):
                        continue  # provably satisfied self-wait
                    kept.append(wv)
                if len(kept) != len(waits):
                    ins.sync_info = mybir.SyncInfo(
                        on_wait=kept, on_update=list(si.on_update or [])
                    )
            si = ins.sync_info
            for up in si.on_update or []:
                if up.update_mode == "sem-inc":
                    cum[up.id] = cum.get(up.id, 0) + up.update_value

    # pass 2: NoOp-split anything still multi-wait (the Drain)
    nop_idx = 0
    for bi, bb in enumerate(blocks):
        for ins in bb.instructions:
            si = getattr(ins, "sync_info", None)
            if si is None:
                continue
            waits = list(si.on_wait or [])
            if len(waits) <= 1:
                continue
            assert bi > 0, f"multi-wait instruction in first block: {ins.name}"
            for other in bb.instructions:
                if other.name == ins.name:
                    break
                assert other.engine != ins.engine, (
                    f"cannot NoOp-split mid-block instruction {ins.name}"
                )
            prev_bb = blocks[bi - 1]
            for wv in waits[:-1]:
                nop = mybir.InstNoOp(name=f"ant-waitsplit-{nop_idx}")
                nop_idx += 1
                nop.engine = ins.engine
                nop.sync_info = mybir.SyncInfo(on_wait=[wv], on_update=[])
                prev_bb.add_instruction(nop)
            ins.sync_info = mybir.SyncInfo(
                on_wait=[waits[-1]], on_update=list(si.on_update or [])
            )


def _numpy_fallback(x, w1, b1, w2, b2, wg, bgv):
    B = x.shape[0]
    R = N_GENES * N_TECH
    xr = x.reshape(B, R).T.astype(np.float32)
    h = np.maximum(xr[:, :, None] * w1[:, None, :] + b1[:, None, :], 0.0)
    s = np.maximum(np.einsum("rbe,re->rb", h, w2) + b2[:, None], 0.0)
    s = s.T.reshape(B, N_TECH, N_GENES)
    out = np.maximum(np.einsum("btg,gt->bg", s, wg) + bgv, 0.0)
    return out.astype(np.float32)


def kernel(x, weights1, bias1, weights2, bias2, weights_g, bias_g):
    global LAST_EXEC_NS, LAST_RESULTS
    x = np.asarray(x, dtype=np.float32)
    w1 = np.asarray(weights1, dtype=np.float32)
    b1 = np.asarray(bias1, dtype=np.float32)
    w2 = np.asarray(weights2, dtype=np.float32)
    b2 = np.asarray(bias2, dtype=np.float32)
    wg = np.asarray(weights_g, dtype=np.float32)
    bgv = np.asarray(bias_g, dtype=np.float32)

    if np.any(b1 != 0.0):
        # hinge-folding below needs b1 == 0; exact general fallback
        return _numpy_fallback(x, w1, b1, w2, b2, wg, bgv)

    G = N_GENES
    # fold the E=4 expand/shrink into two per-row coefficients
    c = (w2 * np.abs(w1)).sum(axis=1)           # [R]
    d = (w2 * np.minimum(w1, 0.0)).sum(axis=1)  # [R]
    ct = np.stack([c[:G], c[G:]], axis=1)       # [G, T]
    dt_ = np.stack([d[:G], d[G:]], axis=1)
    b2t = np.stack([b2[:G], b2[G:]], axis=1)

    awg = np.abs(wg)
    # fold |wg| so the mid relu yields v_t = |wg_t| * s_t
    cp = awg * ct                                # [G, T]
    dp = awg * dt_
    bp = awg * b2t

    pos = wg >= 0.0
    both_pos = pos[:, 0] & pos[:, 1]
    both_neg = (~pos[:, 0]) & (~pos[:, 1])
    mixed = ~(both_pos | both_neg)
    gA = np.nonzero(both_pos)[0]
    gB = np.nonzero(mixed)[0]
    gC = np.nonzero(both_neg & (bgv <= 0.0))[0]
    gD = np.nonzero(both_neg & (bgv > 0.0))[0]

    # per-bucket global tile counts, padded to a multiple of N_CORES
    def n_tiles(n):
        t = -(-n // P)
        return -(-t // N_CORES) * N_CORES if n else 0

    TA, TB, TD = n_tiles(len(gA)), n_tiles(len(gB)), n_tiles(len(gD))
    kinds_pc = ("A",) * (TA // N_CORES) + ("B",) * (TB // N_CORES) + (
        "D",) * (TD // N_CORES)
    NT = len(kinds_pc)

    # global slot table: per bucket, genes padded with -1 to TA*P slots,
    # then chunked per core (core i takes tiles [i*TA/8, (i+1)*TA/8) etc.)
    def pad_slots(g, T):
        s = np.full(T * P, -1, dtype=np.int64)
        s[: len(g)] = g
        return s.reshape(N_CORES, -1) if T else s.reshape(N_CORES, 0)

    slots = np.concatenate(
        [pad_slots(gA, TA), pad_slots(gB, TB), pad_slots(gD, TD)], axis=1
    )  # [N_CORES, NT*P]

    # tech order: slot0 must hold the + tech (identity except mixed genes
    # whose tech0 weight is negative)
    t0 = np.where(mixed & ~pos[:, 0], 1, 0)     # [G]
    t1 = 1 - t0
    tord = np.stack([t0, t1], axis=1)           # [G, T]

    # x -> [G, T, B] fp16 view for gathering
    xt_full = np.ascontiguousarray(x.transpose(2, 1, 0))  # [G, T, B] f32

    idx = np.arange(P)
    in_maps = []
    for i in range(N_CORES):
        gs = slots[i]                            # [NT*P]
        valid = gs >= 0
        gsafe = np.where(valid, gs, 0)
        to = tord[gsafe]                         # [NT*P, T]
        to[~valid] = 0

        xi = np.take_along_axis(
            xt_full[gsafe], to[:, :, None], axis=1
        )                                        # [NT*P, T, B]
        xi[~valid] = 0.0
        xi = xi.reshape(NT, P, 2 * FD).astype(np.float16)

        cpi = np.take_along_axis(cp[gsafe], to, axis=1)   # [NT*P, T]
        dpi = np.take_along_axis(dp[gsafe], to, axis=1)
        bpi = np.take_along_axis(bp[gsafe], to, axis=1)
        bgi = bgv[gsafe].copy()
        for a in (cpi, dpi, bpi):
            a[~valid] = 0.0
        bgi[~valid] = 0.0

        wi = np.zeros((NT, P, NCOL), dtype=np.float32)
        wi[:, :, 0] = bgi.reshape(NT, P)
        wi[:, :, 1] = bpi[:, 0].reshape(NT, P)
        wi[:, :, 2] = bpi[:, 1].reshape(NT, P)
        wi = np.ascontiguousarray(
            wi.transpose(1, 0, 2).reshape(P, NT * NCOL)
        )

        # diagonal stationaries [NT, 4, P(k), P(m)] -> [P, NT*4*P]
        coef = np.stack(
            [cpi[:, 0], dpi[:, 0], cpi[:, 1], dpi[:, 1]], axis=1
        ).reshape(NT, P, 4)
        dgi = np.zeros((NT, 4, P, P), dtype=np.float16)
        for k in range(4):
            dgi[:, k, idx, idx] = coef[:, :, k]
        dgi = np.ascontiguousarray(
            dgi.transpose(2, 0, 1, 3).reshape(P, NT * 4 * P)
        )
        in_maps.append({"x": xi, "w": wi, "dg": dgi})

    has_bias = bool(np.any(bp != 0.0))

    if os.environ.get("KERNEL_NUMPY"):
        # emulate the device dataflow (fp16 rounding included) to validate
        # the host prep without hardware
        results = []
        for i in range(N_CORES):
            xi = in_maps[i]["x"].astype(np.float32).reshape(NT * P, 2, FD)
            wi = in_maps[i]["w"].reshape(P, NT, NCOL).transpose(1, 0, 2)
            dgi = (
                in_maps[i]["dg"].astype(np.float32)
                .reshape(P, NT, 4, P).transpose(1, 2, 0, 3)
            )  # [NT, 4, Pk, Pm]
            coef = dgi[:, :, idx, idx].transpose(0, 2, 1)  # [NT, P, 4]
            coef = coef.reshape(NT * P, 4)
            pfi = np.maximum(xi, 0.0).astype(np.float16).astype(np.float32)
            u0 = coef[:, 0:1] * pfi[:, 0] + coef[:, 1:2] * xi[:, 0]
            u1 = coef[:, 2:3] * pfi[:, 1] + coef[:, 3:4] * xi[:, 1]
            bsl = wi[:, :, 1:3].reshape(NT * P, 2)
            v0 = np.maximum(u0 + bsl[:, 0:1], 0.0).astype(np.float16).astype(np.float32)
            v1 = np.maximum(u1 + bsl[:, 1:2], 0.0).astype(np.float16).astype(np.float32)
            bgl = wi[:, :, 0].reshape(NT * P, 1)
            o = np.zeros_like(v0)
            for jt, kd in enumerate(kinds_pc):
                sl = slice(jt * P, (jt + 1) * P)
                if kd == "A":
                    t = v0[sl] + v1[sl]
                elif kd == "B":
                    t = v0[sl] - v1[sl]
                else:
                    t = -v0[sl] - v1[sl]
                o[sl] = np.maximum(t + bgl[sl], 0.0)
            results.append({"out": o.astype(np.float16)})

        class _R:
            pass

        res = _R()
        res.results = results
        res.exec_time_ns = None
    else:
        key = (kinds_pc, has_bias)
        if key not in _nc_cache:
            _nc_cache[key] = _build_nc(kinds_pc, has_bias)
        nc = _nc_cache[key]

        from concourse.bass_utils import run_bass_kernel_spmd

        trace = bool(int(os.environ.get("KERNEL_TRACE", "0")))
        res = run_bass_kernel_spmd(nc, in_maps, core_ids=list(range(N_CORES)),
                                   trace=trace)
    LAST_EXEC_NS = res.exec_time_ns
    LAST_RESULTS = res

    # assemble: [G, B] then transpose; C-bucket genes and padding stay 0
    outT = np.zeros((G, BATCH), dtype=np.float32)
    for i in range(N_CORES):
        flat = np.asarray(res.results[i]["out"]).reshape(NT * P, BATCH)
        gs = slots[i]
        valid = gs >= 0
        outT[gs[valid]] = flat[valid]
    return np.ascontiguousarray(outT.T)
